# revision 23
# baseline (speedup 1.0000x reference)
"""CubeAttention Trainium2 Bass kernel (8-core SPMD), v2 — bf16 rewrite.

Strategy (per sharding hint): data-parallel over the query grid. The 20^3
query grid is split into 8 slabs: 4 blocks along i (x4) times 2 halves
along j (x2). Each core receives a haloed, host-transposed slice of the
padded spatial embeddings plus replicated weights/tables, computes its
[5,10,20,64] output slice fully on-device, and the host reassembles.

v2 design notes (vs the fp32 v1):
  - All matmuls run in bf16 (1 cycle/col + fast-weight-load vs fp32's
    4 cycles/col and doubled instruction count); PSUM accumulation stays
    fp32. Verified in numpy: rel l2 ~9e-3 vs the 2e-2 gate.
  - Logits are computed TRANSPOSED, per 128-row support chunk:
    psLT[s,q] = kp_chunk^T @ qaug. exp() evictions then land S'^T in
    exactly the layout the AV matmul wants as its moving operand, so the
    18 PE transposes per block of v1 disappear entirely.
  - V support rows are produced position-major by using the gathered
    (channel-major) se support as the matmul WEIGHTS: out[pos,ch] =
    sgath_chunk^T @ Wvnb. The AV stationary operand is [V | ones | ind27]
    so one accumulation group yields values, Z (softmax denominator) and
    the 27 axis-marginals used for the separable relpos-value fixup.
  - The relpos C-row and value-fixup matmuls are batched across all 8
    blocks (block-major query layout makes every (axis, coord) query
    subset a regular <=4D access pattern): 30 N=100 matmuls replace
    240 N=25 ones.
  - Softmax runs unnormalized; 1/Z is applied per-query as the ACT scale
    after the output projection (queries are then the partition dim).
    Z reaches partition-major form via one reordering DMA.
  - Biases: bk cancels in softmax (constant per query); bv and bo fold
    into one host-computed constant (bv@Wo+bo) added at the end; bq is
    applied on the Q projection eviction.
"""

import numpy as np

SCOPE, GN, D, CAP = 2, 20, 64, 32
S3 = 729
NEG = np.float32(-1e9)

# wpW bf16 pack [64, 1255] column offsets
_WP = dict(Wq=0, Wknb=64, Wvnb=128, Wo=192, Zki=256, Zkj=269, Zkk=282,
           rvsh=295)  # rvsh: 15 tables of 64 cols: 295 + 64*(5*ax+g)
_WPW_COLS = 295 + 15 * 64  # 1255

_CACHE = {}


def _bass_mod(reps=1, dbg=False):
    key = ("nc", reps, dbg)
    if key in _CACHE:
        return _CACHE[key]
    import sys
    for p in ("/opt/trn_rl_repo", "/root/.axon_site/_ro/trn_rl_repo"):
        if p not in sys.path:
            sys.path.append(p)
    import concourse.tile as tile
    from concourse import bacc, mybir

    f32 = mybir.dt.float32
    bf16 = mybir.dt.bfloat16
    AF = mybir.ActivationFunctionType

    nc = bacc.Bacc("TRN2", target_bir_lowering=False, debug=False)
    P = {}
    P["seT"] = nc.declare_dram_parameter("seT", [64, 3024], bf16, isOutput=False)
    P["wpW"] = nc.declare_dram_parameter("wpW", [64, _WPW_COLS], bf16,
                                         isOutput=False)
    P["wp32"] = nc.declare_dram_parameter("wp32", [125, 65], f32, isOutput=False)
    P["masks3"] = nc.declare_dram_parameter("masks3", [9, 3000], f32,
                                            isOutput=False)
    P["ind27"] = nc.declare_dram_parameter("ind27", [27, 729], bf16,
                                           isOutput=False)
    P["indV"] = nc.declare_dram_parameter("indV", [128, 384], bf16,
                                          isOutput=False)
    out_p = nc.declare_dram_parameter("out", [8, 125, 64], f32, isOutput=True)
    if dbg:
        D = {}
        D["KPT"] = nc.declare_dram_parameter("dKPT", [64, 3024], bf16,
                                             isOutput=True)
        D["Qall"] = nc.declare_dram_parameter("dQall", [64, 1000], bf16,
                                              isOutput=True)
        D["CallMs"] = nc.declare_dram_parameter("dCallMs", [27, 1000], bf16,
                                                isOutput=True)
        D["AVall"] = nc.declare_dram_parameter("dAVall", [64, 1000], f32,
                                               isOutput=True)
        D["M"] = nc.declare_dram_parameter("dM", [3, 9, 1000], bf16,
                                           isOutput=True)
        D["ZT"] = nc.declare_dram_parameter("dZT", [125, 8], f32,
                                            isOutput=True)

    # support chunking: 6 chunks of <=128 of the 729 support positions
    CH = [(128 * c, min(128, 729 - 128 * c)) for c in range(6)]

    with tile.TileContext(nc) as tc:
        with (
            tc.tile_pool(name="const", bufs=1) as const,
            tc.tile_pool(name="work", bufs=3) as work,
            tc.tile_pool(name="psA", bufs=2, space="PSUM") as psA,
            tc.tile_pool(name="psQK", bufs=1, space="PSUM") as psQK,
            tc.tile_pool(name="psV", bufs=2, space="PSUM") as psV,
            tc.tile_pool(name="psAV", bufs=1, space="PSUM") as psAV,
        ):
            # ---- constants ----
            wp = const.tile([64, _WPW_COLS], bf16, tag="wpW")
            nc.sync.dma_start(wp[:], P["wpW"][:])
            sb = {
                "Wq": wp[0:64, 0:64], "Wknb": wp[0:64, 64:128],
                "Wvnb": wp[0:64, 128:192], "Wo": wp[0:64, 192:256],
                "Zki": wp[0:64, 256:269], "Zkj": wp[0:64, 269:282],
                "Zkk": wp[0:64, 282:295],
            }
            Zk = {"i": sb["Zki"], "j": sb["Zkj"], "k": sb["Zkk"]}

            def rvsh(ax, g):
                o = _WP["rvsh"] + 64 * (5 * ax + g)
                return wp[0:9, o:o + 64]

            wp32 = const.tile([125, 65], f32, tag="wp32")
            nc.scalar.dma_start(wp32[:], P["wp32"][:])
            bq_ap = wp32[0:64, 0:1]
            bfin = wp32[0:125, 1:65]

            masks3 = const.tile([9, 3000], f32, tag="masks3")
            nc.gpsimd.dma_start(masks3[:], P["masks3"][:])

            seT = const.tile([64, 3024], bf16, tag="seT")
            nc.sync.dma_start(seT[:], P["seT"][:])
            seT4 = seT[:].rearrange("p (i j k) -> p i j k", i=9, j=14, k=24)

            # kp tiles (3x ping-pong): rows 0:64 KP gather, rows 64:91 ind27
            kpa = []
            for t in range(3):
                kt = const.tile([91, 729], bf16, tag=f"kpa{t}")
                nc.scalar.dma_start(kt[64:91, :], P["ind27"][:])
                kpa.append(kt)

            # Vaug tiles (3 sets x 6 chunks) [128,128]:
            #   cols 0:64 V, 64 ones, 65:96 zero, 96:123 ind27, 123:128 zero
            # S'T tiles (3 sets x 6 chunks) [128,125]
            vaug, stt = {}, {}
            for s in range(3):
                for c in range(6):
                    vt = const.tile([128, 128], bf16, tag=f"vaug{s}_{c}")
                    nc.vector.memset(vt[:], 0.0)
                    nc.sync.dma_start(vt[:, 64:128], P["indV"][:, 64 * c:64 * c + 64])
                    vaug[(s, c)] = vt
                    st = const.tile([128, 125], bf16, tag=f"st{s}_{c}")
                    nc.gpsimd.memset(st[:], 0.0)
                    stt[(s, c)] = st

            # persistent accumulators / staging
            KPT = const.tile([64, 3024], bf16, tag="KPT")
            Qall = const.tile([64, 1000], bf16, tag="Qall")
            CallMs = const.tile([27, 1000], bf16, tag="CallMs")
            Max = {}
            for ax in range(3):
                mt = const.tile([9, 1000], bf16, tag=f"M{ax}")
                Max[ax] = mt
            AVall = const.tile([64, 1000], f32, tag="AVall")
            ZallT = const.tile([125, 8], f32, tag="ZallT")
            rzT = const.tile([125, 8], f32, tag="rzT")
            avf = []
            for t in range(2):
                a = const.tile([64, 128], bf16, tag=f"avf{t}")
                nc.vector.memset(a[:], 0.0)
                avf.append(a)

            # ---- phase 0: KPT, Qall, C rows ----
            for c in range(6):
                sl = slice(504 * c, 504 * (c + 1))
                ps = psA.tile([64, 512], f32, tag="big")
                nc.tensor.matmul(ps[:, 0:504], sb["Wknb"], seT[:, sl],
                                 start=True, stop=True)
                nc.scalar.activation(KPT[:, sl], ps[:, 0:504], AF.Identity)
            KPT4 = KPT[:].rearrange("p (i j k) -> p i j k", i=9, j=14, k=24)

            for blk in range(8):
                bj, bk = blk // 4, blk % 4
                ps = psQK.tile([64, 128], f32, tag="plA", bufs=2)
                rhs = seT4[:, 2:7, 2 + 5 * bj:7 + 5 * bj, 2 + 5 * bk:7 + 5 * bk]
                nc.tensor.matmul(ps[:, 0:125], sb["Wq"], rhs,
                                 start=True, stop=True)
                nc.scalar.activation(Qall[:, 125 * blk:125 * blk + 125],
                                     ps[:, 0:125], AF.Identity, bias=bq_ap)

            # C rows: for (ax, g) batched over blocks; query col layout
            # q = 125*blk + 25a + 5b + c. Views (all <=4D):
            Qv_i = Qall[:].rearrange("p (blk a bc) -> p blk a bc", blk=8, a=5)
            Qv_j = Qall[:].rearrange("p (ba b c) -> p ba b c", ba=40, b=5)
            Qv_k = Qall[:].rearrange("p (bb c) -> p bb c", bb=200, c=5)
            for ax in range(3):
                axn = "ijk"[ax]
                for h in range(2):
                    ps = psA.tile([9, 512], f32, tag="big")
                    pv_i = ps[:, 0:500].rearrange(
                        "p (blk a bc) -> p blk a bc", blk=4, a=5)
                    pv_j = ps[:, 0:500].rearrange(
                        "p (ba b c) -> p ba b c", ba=20, b=5)
                    pv_k = ps[:, 0:500].rearrange("p (bb c) -> p bb c", bb=100)
                    for g in range(5):
                        lhsT = Zk[axn][:, 4 - g:13 - g]
                        if ax == 0:
                            rhs = Qv_i[:, 4 * h:4 * h + 4, g, :]
                            o = pv_i[:, :, g, :]
                        elif ax == 1:
                            rhs = Qv_j[:, 20 * h:20 * h + 20, g, :]
                            o = pv_j[:, :, g, :]
                        else:
                            rhs = Qv_k[:, 100 * h:100 * h + 100, g]
                            o = pv_k[:, :, g]
                        nc.tensor.matmul(o, lhsT, rhs, start=True, stop=True)
                    cst = work.tile([9, 512], bf16, tag="cstg")
                    nc.vector.tensor_add(
                        cst[:, 0:500], ps[:, 0:500],
                        masks3[:, 1000 * ax + 500 * h:1000 * ax + 500 * h + 500])
                    nc.sync.dma_start(
                        CallMs[9 * ax:9 * ax + 9, 500 * h:500 * h + 500],
                        cst[:, 0:500])

            # ---- block loop ----
            def stage_A(blk):
                b8 = blk % 8
                s = b8 % 3
                bj, bk = b8 // 4, b8 % 4
                jsl = slice(5 * bj, 5 * bj + 9)
                ksl = slice(5 * bk, 5 * bk + 9)
                kp = kpa[s]
                nc.vector.tensor_copy(
                    kp[0:64, :].rearrange("p (a b c) -> p a b c", a=9, b=9),
                    KPT4[:, :, jsl, ksl])
                sg = work.tile([64, 729], bf16, tag="sgath")
                nc.gpsimd.tensor_copy(
                    sg[:].rearrange("p (a b c) -> p a b c", a=9, b=9),
                    seT4[:, :, jsl, ksl])
                qa = work.tile([91, 125], bf16, tag="qa")
                nc.vector.tensor_copy(qa[0:64, :],
                                      Qall[:, 125 * b8:125 * b8 + 125])
                nc.vector.tensor_copy(qa[64:91, :],
                                      CallMs[:, 125 * b8:125 * b8 + 125])
                pv = psV.tile([128, 384], f32, tag="pv")
                for c, (o0, cn) in enumerate(CH):
                    nc.tensor.matmul(pv[0:cn, 64 * c:64 * c + 64],
                                     sg[:, o0:o0 + cn], sb["Wvnb"],
                                     start=True, stop=True)
                    if c % 2 == 0:
                        nc.vector.tensor_copy(vaug[(s, c)][0:cn, 0:64],
                                              pv[0:cn, 64 * c:64 * c + 64])
                    else:
                        nc.scalar.copy(vaug[(s, c)][0:cn, 0:64],
                                       pv[0:cn, 64 * c:64 * c + 64])
                return (blk, kp, qa)

            def stage_B(st):
                blk, kp, qa = st
                b8 = blk % 8
                s = b8 % 3
                qsl = slice(125 * b8, 125 * b8 + 125)
                plA = psQK.tile([128, 512], f32, tag="plA", bufs=2)
                plB = psQK.tile([128, 256], f32, tag="plB", bufs=1)
                sts = []
                for c, (o0, cn) in enumerate(CH):
                    pl = (plA[0:cn, 128 * c:128 * c + 125] if c < 4
                          else plB[0:cn, 128 * (c - 4):128 * (c - 4) + 125])
                    nc.tensor.matmul(pl, kp[:, o0:o0 + cn], qa[:],
                                     start=True, stop=True)
                    nc.scalar.activation(stt[(s, c)][0:cn, :], pl, AF.Exp)
                    sts.append(stt[(s, c)])
                psv = psAV.tile([128, 128], f32, tag="psv")
                for c in range(6):
                    nc.tensor.matmul(psv[:, 0:125], vaug[(s, c)], sts[c][:],
                                     start=(c == 0), stop=(c == 5))
                nc.vector.tensor_copy(AVall[:, qsl], psv[0:64, 0:125])
                zst = work.tile([1, 125], f32, tag="zst")
                nc.vector.tensor_copy(zst[:], psv[64:65, 0:125])
                nc.scalar.dma_start(ZallT[:, b8:b8 + 1], zst[:])
                mst = work.tile([27, 125], bf16, tag="mstg")
                nc.vector.tensor_copy(mst[:], psv[96:123, 0:125])
                for ax in range(3):
                    nc.sync.dma_start(Max[ax][:, qsl], mst[9 * ax:9 * ax + 9, :])

            def tail(rep):
                nc.vector.reciprocal(rzT[:], ZallT[:])
                # fixups: psF halves [64, 500]
                for h in range(2):
                    ps = psA.tile([64, 512], f32, tag="big")
                    pv_i = ps[:, 0:500].rearrange(
                        "p (blk a bc) -> p blk a bc", blk=4, a=5)
                    pv_j = ps[:, 0:500].rearrange(
                        "p (ba b c) -> p ba b c", ba=20, b=5)
                    pv_k = ps[:, 0:500].rearrange("p (bb c) -> p bb c", bb=100)
                    Mv_i = Max[0][:].rearrange(
                        "p (blk a bc) -> p blk a bc", blk=8, a=5)
                    Mv_j = Max[1][:].rearrange(
                        "p (ba b c) -> p ba b c", ba=40, b=5)
                    Mv_k = Max[2][:].rearrange("p (bb c) -> p bb c", bb=200)
                    for ax in range(3):
                        for g in range(5):
                            lhsT = rvsh(ax, g)
                            if ax == 0:
                                rhs = Mv_i[:, 4 * h:4 * h + 4, g, :]
                                o = pv_i[:, :, g, :]
                            elif ax == 1:
                                rhs = Mv_j[:, 20 * h:20 * h + 20, g, :]
                                o = pv_j[:, :, g, :]
                            else:
                                rhs = Mv_k[:, 100 * h:100 * h + 100, g]
                                o = pv_k[:, :, g]
                            # start=True marks the whole PSUM bank pending-
                            # zero, so only the very first matmul may set it:
                            # later writes to still-pending columns overwrite
                            # (ax=0 covers every column), then ax=1/2 add.
                            nc.tensor.matmul(o, lhsT, rhs,
                                             start=(ax == 0 and g == 0),
                                             stop=(ax == 2 and g == 4))
                    for m in range(4):
                        b8 = 4 * h + m
                        av = avf[b8 % 2]
                        nc.vector.tensor_add(
                            av[:, 0:125],
                            AVall[:, 125 * b8:125 * b8 + 125],
                            ps[:, 125 * m:125 * m + 125])
                        po = psAV.tile([128, 64], f32, tag="psv", bufs=1)
                        nc.tensor.matmul(po[:], av[:], sb["Wo"],
                                         start=True, stop=True)
                        osb = work.tile([125, 64], f32, tag="osb")
                        nc.scalar.activation(osb[:], po[0:125, :], AF.Identity,
                                             scale=rzT[:, b8:b8 + 1])
                        nc.vector.tensor_add(osb[:], osb[:], bfin)
                        nc.sync.dma_start(out_p[b8], osb[:])

            if dbg:
                def dump():
                    nc.sync.dma_start(D["KPT"][:], KPT[:])
                    nc.sync.dma_start(D["Qall"][:], Qall[:])
                    nc.sync.dma_start(D["CallMs"][:], CallMs[:])
                    nc.sync.dma_start(D["AVall"][:], AVall[:])
                    for ax in range(3):
                        nc.sync.dma_start(D["M"][ax], Max[ax][:])
                    nc.sync.dma_start(D["ZT"][:], ZallT[:])

            from collections import deque
            for rep in range(reps):
                pending = deque()
                for blk in range(8):
                    pending.append(stage_A(blk))
                    if len(pending) > 2:
                        stage_B(pending.popleft())
                while pending:
                    stage_B(pending.popleft())
                tail(rep)
                if dbg:
                    dump()

    nc.compile()
    _CACHE[key] = nc
    _CACHE["nc"] = nc
    return nc


def _host_tables():
    if "tables" in _CACHE:
        return _CACHE["tables"]
    from ml_dtypes import bfloat16
    s = np.arange(S3)
    si, sj, sk = s // 81, (s // 9) % 9, s % 9
    ind27 = np.zeros((27, S3), np.float32)
    for t in range(9):
        ind27[t] = (si == t)
        ind27[9 + t] = (sj == t)
        ind27[18 + t] = (sk == t)
    # indV: per support chunk c, cols 64c..64c+64 of Vaug consts:
    #   local col 0 = ones, 32:59 = ind27 (psv rows 96:123 <-> Vaug 96:123)
    indV = np.zeros((128, 384), np.float32)
    for c in range(6):
        n = min(128, 729 - 128 * c)
        rows = np.arange(n)
        svals = 128 * c + rows
        indV[rows, 64 * c + 0] = 1.0
        for t in range(9):
            indV[rows, 64 * c + 32 + t] = (svals // 81 == t)
            indV[rows, 64 * c + 41 + t] = ((svals // 9) % 9 == t)
            indV[rows, 64 * c + 50 + t] = (svals % 9 == t)
    _CACHE["tables"] = (ind27.astype(bfloat16), indV.astype(bfloat16))
    return _CACHE["tables"]


def _masks_for_core(bi, h):
    q = np.arange(125)
    a, b, c = q // 25, (q // 5) % 5, q % 5
    sig = np.arange(9)[:, None]

    def vmask(qx, off):
        return (qx + off > 2) & (qx + off < 22)

    out = np.zeros((8, 3, 9, 125), np.float32)
    for blk in range(8):
        bj, bkk = blk // 4, blk % 4
        qi = 5 * bi + a
        qj = 10 * h + 5 * bj + b
        qk = 5 * bkk + c
        oi = sig - a[None, :]
        oj = sig - b[None, :]
        ok = sig - c[None, :]
        wi = (oi >= 0) & (oi <= 4)
        wj = (oj >= 0) & (oj <= 4)
        wk = (ok >= 0) & (ok <= 4)
        out[blk, 0] = np.where(wi & vmask(qj[None, :], oi), 0.0, NEG)
        out[blk, 1] = np.where(wj & vmask(qi[None, :], oj), 0.0, NEG)
        out[blk, 2] = np.where(wk & vmask(qk[None, :], ok), 0.0, NEG)
    # masks3 [9, 3000]: masks3[t, 1000*ax + 125*blk + q]
    m3 = np.zeros((9, 3000), np.float32)
    for blk in range(8):
        for ax in range(3):
            m3[:, 1000 * ax + 125 * blk:1000 * ax + 125 * blk + 125] = \
                out[blk, ax]
    return m3


def _pack_weights(inputs):
    from ml_dtypes import bfloat16
    Wk, Wv = inputs["Wk"], inputs["Wv"]
    r = inputs["relpos_w"]
    wp = np.zeros((64, _WPW_COLS), np.float32)

    def put(off, arr):
        rr, cc = arr.shape
        wp[0:rr, off:off + cc] = arr

    put(_WP["Wq"], inputs["Wq"])
    put(_WP["Wknb"], Wk[96:160])
    put(_WP["Wvnb"], Wv[96:160])
    put(_WP["Wo"], inputs["Wo"])
    # Zk (key relpos, axis->weight mapping per validated v1): i->Wk[32:64],
    # j->Wk[0:32], k->Wk[64:96]; padded [64,13] with G^T in cols 4:9
    for off, blkr in ((_WP["Zki"], Wk[32:64]), (_WP["Zkj"], Wk[0:32]),
                      (_WP["Zkk"], Wk[64:96])):
        G = r @ blkr  # [5, 64]
        z = np.zeros((64, 13), np.float32)
        z[:, 4:9] = G.T
        put(off, z)
    # RVSH: i->Wv[32:64], j->Wv[0:32], k->Wv[64:96]
    RVs = [r @ Wv[32:64], r @ Wv[0:32], r @ Wv[64:96]]
    for ax in range(3):
        for g in range(5):
            t = np.zeros((9, 64), np.float32)
            lo, hi = g, min(9, g + 5)
            t[lo:hi] = RVs[ax][0:hi - lo]
            put(_WP["rvsh"] + 64 * (5 * ax + g), t)
    return wp.astype(bfloat16)


def _pack_w32(inputs):
    wp = np.zeros((125, 65), np.float32)
    wp[0:64, 0] = inputs["bq"]
    bfin = inputs["bv"] @ inputs["Wo"] + inputs["bo"]
    wp[0:125, 1:65] = np.broadcast_to(bfin, (125, 64))
    return wp


def _make_in_maps(inputs):
    from ml_dtypes import bfloat16
    se = np.asarray(inputs["spatial_embeddings"], np.float32)
    inputs = {k: np.asarray(v, np.float32) for k, v in inputs.items()}
    ind27, indV = _host_tables()
    se_pad = np.pad(se, ((2, 2),) * 3 + ((0, 0),))
    shared = dict(wpW=_pack_weights(inputs), wp32=_pack_w32(inputs),
                  ind27=ind27, indV=indV)
    in_maps = []
    for core in range(8):
        bi, h = core // 2, core % 2
        slab = se_pad[5 * bi:5 * bi + 9, 10 * h:10 * h + 14, :, :]
        m = dict(shared)
        m["seT"] = np.ascontiguousarray(
            slab.transpose(3, 0, 1, 2)).reshape(64, 3024).astype(bfloat16)
        m["masks3"] = _masks_for_core(bi, h)
        in_maps.append(m)
    return in_maps


def _assemble(results):
    out = np.empty((20, 20, 20, 64), np.float32)
    for core in range(8):
        bi, h = core // 2, core % 2
        blocks = np.asarray(results[core]["out"]).reshape(8, 5, 5, 5, 64)
        for blk in range(8):
            bj, bkk = blk // 4, blk % 4
            out[5 * bi:5 * bi + 5,
                10 * h + 5 * bj:10 * h + 5 * bj + 5,
                5 * bkk:5 * bkk + 5] = blocks[blk]
    return out


def kernel(**inputs):
    import sys
    for pth in ("/opt/trn_rl_repo", "/root/.axon_site/_ro/trn_rl_repo"):
        if pth not in sys.path:
            sys.path.append(pth)
    from concourse.bass_utils import run_bass_kernel_spmd

    nc = _bass_mod()
    in_maps = _make_in_maps(inputs)
    res = run_bass_kernel_spmd(nc, in_maps, core_ids=list(range(8)))
    return _assemble(res.results)
